# revision 22
# baseline (speedup 1.0000x reference)
"""TopoEncoder Trainium2 kernel (8 NeuronCores, data-parallel over batch).

Pipeline per core (64 samples):
  1. DMA x-shard (two HWDGE queues, 3.2KB descriptors), mean over T
     (DVE+GpSimd add-tree, PE pair-matrix fold)
  2. pairwise channel-L2 distance matrix d [64,25,25]
  3. local max -> PE transpose -> AllReduce(max) across the 8 cores,
     triggered immediately. Its latency is dominated by the CC-stream
     entry barrier (~54us) + ~23us fixed op cost; everything through the
     top-24 extraction hides under it. Global min is NOT communicated:
     the distance-matrix diagonal is exactly sqrt(1e-12) = 1e-6 on every
     sample, so global min == 1e-6 always.
  4. Floyd-Warshall min-max closure M (25 steps) -> MST mask = (M >= d)
     (0-dim persistence deaths = MST edge weight multiset; downstream
      structure-element sum is permutation-invariant, so order is free)
  5. top-24 extraction of masked upper-tri values (max8 + match_replace)
  6. structure-element layer via a single K=49 PE contraction:
       s[b,(e,p)] = inv^2*u_e*w'^2[b,p] - 2*inv*u_e*c2_e*w'[b,p] + u_e*c2_e^2
                  = u_e * ((w'[b,p] * inv) - c2_e)^2,   w' = deaths - 1e-6
     The death-index selector (delta mask) and parameter rows are baked
     into a [49, E*NT] rhs during the DMA-wait head; post-AllReduce work
     is only: reciprocal, two [24,64] row scales, 4 chunked PE matmuls,
     exp (Scalar), and per-e reduce (DVE) -> ~5us tail.

All AR-dependent prep runs on GpSimd/PE/Scalar or is forced late via
tile_wait_until so the Vector queue never blocks on the collective.

All constant broadcasts across partitions are built with K=1 PE matmuls
(ones[1,B] (x) row) — partition-broadcast DMAs on SWDGE are ~7x slower
per packet and starve the big x transfer.
"""

from contextlib import ExitStack

import numpy as np

import bass_rust
import concourse.bass as bass
import concourse.tile as tile
from concourse import mybir
from concourse.bass_utils import run_bass_kernel_spmd

N_CORES = 8
B = 64          # samples per core
C, T, V, E = 3, 128, 25, 64
VV = V * V
NT = V - 1      # deaths per sample (24)
K2 = 64          # PE contraction depth for the structure-element layer
ROW_Q = 56       # rhs row carrying the constant (u*c2^2) term
DT = mybir.dt.float32
GMIN = 1e-6     # exact global min of d: the diagonal is sqrt(1e-12)
ECH = 16        # e-channels per tail chunk (PE/exp/reduce pipeline)


def _split_excess_waits(nc, cap=1):
    """The walrus build in this env rejects instructions carrying more than
    ~2 semaphore-wait commands. Move excess waits onto same-engine NOPs
    inserted immediately before the offending instruction."""
    n_split = 0
    for bb in nc.main_func.blocks:
        insts = bb.instructions
        i = 0
        while i < len(insts):
            ins = insts[i]
            si = ins.sync_info
            waits = list(si.on_wait) if si and si.on_wait else []
            if len(waits) > cap:
                extra, keep = waits[:-cap], waits[-cap:]
                ins.sync_info = mybir.SyncInfo(
                    on_wait=keep, on_update=list(si.on_update or [])
                )
                for j, w in enumerate(extra):
                    nop = bass_rust.InstNoOp(
                        name=f"I-wsplit-{n_split}-{j}",
                        engine=ins.engine,
                        sync_info=mybir.SyncInfo(on_wait=[w], on_update=[]),
                    )
                    insts.insert(i, nop)
                    i += 1
                n_split += 1
            i += 1
    return n_split


def _build_program():
    A = mybir.AluOpType
    ACT = mybir.ActivationFunctionType
    nc = bass.Bass("TRN2", debug=False, num_devices=N_CORES)

    x_in = nc.dram_tensor("x", [B, C, T, V], DT, kind="ExternalInput").ap()
    pm_in = nc.dram_tensor("pm", [128, B], DT, kind="ExternalInput").ap()
    ut_in = nc.dram_tensor("ut", [B, VV], DT, kind="ExternalInput").ap()
    id_in = nc.dram_tensor("id64", [B, B], DT, kind="ExternalInput").ap()
    rhs_in = nc.dram_tensor("rhs", [K2, E * NT], DT, kind="ExternalInput").ap()
    ab_in = nc.dram_tensor("ab", [B, E], DT, kind="ExternalInput").ap()
    cu_in = nc.dram_tensor("cu", [1, 2 * E], DT, kind="ExternalInput").ap()
    out_d = nc.dram_tensor("out", [B, E], DT, kind="ExternalOutput").ap()

    with tile.TileContext(nc, num_cores=N_CORES) as tc, ExitStack() as ctx:
        sb = ctx.enter_context(tc.tile_pool(name="sb", bufs=1))
        psum = ctx.enter_context(tc.tile_pool(name="psum", bufs=1, space="PSUM"))
        dram = ctx.enter_context(tc.tile_pool(name="dram", bufs=1, space="DRAM"))

        # ---- x DMA first: partition p = t2*64 + b, free = (c, t64, v) ----
        # two t64-half tiles so the add-tree overlaps the second half's DMA;
        # both HWDGE queues used; 3.2KB contiguous descriptors
        xa = sb.tile([128, C, T // 4, V], DT)
        xb = sb.tile([128, C, T // 4, V], DT)
        nc.sync.dma_start(xa[0:B], x_in[:, :, 0:32, :])
        nc.scalar.dma_start(xa[96:128], x_in[32:64, :, 64:96, :])
        nc.sync.dma_start(xa[B:96], x_in[0:32, :, 64:96, :])
        nc.sync.dma_start(xb[0:54], x_in[0:54, :, 32:64, :])
        nc.scalar.dma_start(xb[54:B], x_in[54:B, :, 32:64, :])
        nc.scalar.dma_start(xb[B:128], x_in[:, :, 96:128, :])

        # ---- small constant loads (HWDGE, few descriptors) ----
        pm_t = sb.tile([128, B], DT)
        nc.sync.dma_start(pm_t[:], pm_in[:])
        id64 = sb.tile([B, B], DT)
        nc.sync.dma_start(id64[:], id_in[:])
        rhs = sb.tile([K2, E, NT], DT)
        nc.sync.dma_start(rhs[:], rhs_in[:])
        Ab = sb.tile([B, E], DT)
        nc.scalar.dma_start(Ab[:], ab_in[:])
        utb = sb.tile([B, VV], DT)
        nc.scalar.dma_start(utb[:], ut_in[:])
        ones1 = sb.tile([1, B], DT)
        nc.vector.memset(ones1[:], 1.0)
        eps = sb.tile([128, 1], DT)
        nc.vector.memset(eps[:], 1e-12)

        # [c2 | u] broadcast rows for the direct DVE/GpSimd SEL slices
        cu_in_row = sb.tile([1, 2, E], DT)
        nc.scalar.dma_start(cu_in_row[:], cu_in[:])
        bc2 = psum.tile([B, 2, E], DT)
        nc.tensor.matmul(out=bc2[:], lhsT=ones1[:], rhs=cu_in_row[:],
                         start=True, stop=True)
        cu2 = sb.tile([B, 2, E], DT)
        nc.vector.tensor_copy(cu2[:], bc2[:])

        # structure-element lhsT rows: [w'^2.T * inv^2; w'.T * inv; ones]
        # (memset must start at partition 0: set all rows, the first 48 are
        # overwritten by the post-AR scales)
        LT = sb.tile([K2, B], DT)
        nc.vector.memset(LT[:], 1.0)

        # PWP management: load the Sqrt table early (dummy, no deps) so the
        # d-matrix sqrt pays no mid-chain table switch; the Exp table is
        # loaded after dmat (dummy depends on it) so the tail exps are clean
        dumm = sb.tile([1, 2], DT)
        nc.vector.memset(dumm[:], 0.0)
        nc.scalar.sqrt(dumm[:], dumm[:])

        # ---- mean over T: in-place add trees (DVE: c0-c1, GpSimd: c2),
        # then PE pair-matrix fold ----
        for xh in (xa, xb):
            for w in (16, 8, 4, 2, 1):
                nc.vector.tensor_tensor(
                    out=xh[:, 0:2, 0:w, :],
                    in0=xh[:, 0:2, 0:w, :],
                    in1=xh[:, 0:2, w : 2 * w, :],
                    op=A.add,
                )
                nc.gpsimd.tensor_tensor(
                    out=xh[:, 2, 0:w, :],
                    in0=xh[:, 2, 0:w, :],
                    in1=xh[:, 2, w : 2 * w, :],
                    op=A.add,
                )
        nc.vector.tensor_tensor(
            out=xa[:, 0:2, 0:1, :], in0=xa[:, 0:2, 0:1, :], in1=xb[:, 0:2, 0:1, :],
            op=A.add,
        )
        nc.gpsimd.tensor_tensor(
            out=xa[:, 2, 0:1, :], in0=xa[:, 2, 0:1, :], in1=xb[:, 2, 0:1, :],
            op=A.add,
        )
        ps_xm = psum.tile([B, C, V], DT)
        nc.tensor.matmul(out=ps_xm[:], lhsT=pm_t[:], rhs=xa[:, :, 0, :],
                         start=True, stop=True)
        xm = sb.tile([B, C, V], DT)
        nc.vector.tensor_copy(xm[:], ps_xm[:])

        # ---- distance matrix ----
        df = sb.tile([B, C, V, V], DT)
        xmb_i = xm.unsqueeze(-1).broadcast_to([B, C, V, V])
        xmb_j = xm.unsqueeze(2).broadcast_to([B, C, V, V])
        nc.vector.tensor_tensor(
            out=df[:, 0:2], in0=xmb_i[:, 0:2], in1=xmb_j[:, 0:2], op=A.subtract
        )
        nc.gpsimd.tensor_tensor(
            out=df[:, 2], in0=xmb_i[:, 2], in1=xmb_j[:, 2], op=A.subtract
        )
        nc.scalar.square(df[:, 0:2], df[:, 0:2])
        nc.vector.tensor_tensor(out=df[:, 2], in0=df[:, 2], in1=df[:, 2], op=A.mult)
        d2 = sb.tile([B, VV], DT)
        d23 = d2.rearrange("p (i j) -> p i j", i=V)
        nc.vector.tensor_tensor(out=d23[:], in0=df[:, 0], in1=df[:, 1], op=A.add)
        nc.vector.tensor_tensor(out=d23[:], in0=d23[:], in1=df[:, 2], op=A.add)
        dmat = sb.tile([B, VV], DT)
        nc.scalar.activation(dmat[:], d2[:], ACT.Sqrt, bias=eps[0:B, 0:1], scale=1.0)
        # Exp-table preload, pinned after the sqrt via the dmat read
        nc.scalar.activation(dumm[:], dmat[0:1, 0:2], ACT.Exp, bias=0.0, scale=-1.0)

        # ---- local max from d^2 (sqrt commutes with max; the [B,VV] reduce
        # overlaps the Scalar-engine sqrt) -> PE transpose -> AllReduce ----
        lmax = sb.tile([B, 1], DT)
        nc.vector.tensor_reduce(out=lmax[:], in_=d2[:],
                                axis=mybir.AxisListType.X, op=A.max)
        lmaxT = psum.tile([1, B], DT)
        nc.tensor.matmul(out=lmaxT[:], lhsT=lmax[:], rhs=id64[:], start=True, stop=True)
        gmr2 = sb.tile([1, 1], DT)
        nc.vector.tensor_reduce(out=gmr2[:], in_=lmaxT[:],
                                axis=mybir.AxisListType.X, op=A.max)
        gmr = sb.tile([1, 1], DT)
        nc.scalar.activation(gmr[:], gmr2[:], ACT.Sqrt, bias=eps[0:1, 0:1], scale=1.0)
        cin = dram.tile([1, 1], DT)
        cout = dram.tile([1, 1], DT)
        nc.scalar.dma_start(cin[:], gmr[:])
        nc.gpsimd.collective_compute(
            "AllReduce", A.max, replica_groups=[list(range(N_CORES))],
            ins=[cin.opt()], outs=[cout.opt()],
        )
        # AR-dependent scalar prep: rng and 1/rng on the [1,1] scalar FIRST,
        # then one PE matmul broadcasts [inv, inv^2] to all partitions
        gsb = sb.tile([1, 1], DT)
        rng0 = sb.tile([1, 1], DT)
        ivrow = sb.tile([1, 2], DT)
        ivp = psum.tile([B, 2], DT)
        iv2 = sb.tile([B, 2], DT)
        with tc.tile_wait_until(0.068):
            nc.scalar.dma_start(gsb[:], cout[:])
        with tc.tile_wait_until(0.0695):
            nc.vector.tensor_scalar_add(rng0[:], gsb[:], -GMIN)
            nc.vector.reciprocal(ivrow[:, 0:1], rng0[:])
            nc.vector.tensor_tensor(out=ivrow[:, 1:2], in0=ivrow[:, 0:1],
                                    in1=ivrow[:, 0:1], op=A.mult)
            nc.tensor.matmul(out=ivp[:], lhsT=ones1[:], rhs=ivrow[:],
                             start=True, stop=True)
            nc.vector.tensor_copy(iv2[:], ivp[:])
        invb = iv2[:, 0:1]
        inv2b = iv2[:, 1:2]

        # ---- premasked values on GpSimd (idle during FW) ----
        dut = sb.tile([B, VV], DT)
        nc.gpsimd.tensor_tensor(out=dut[:], in0=dmat[:], in1=utb[:], op=A.mult)

        # ---- Floyd-Warshall min-max closure (in place) ----
        M = sb.tile([B, VV], DT)
        M3 = M.rearrange("p (i j) -> p i j", i=V)
        dm3 = dmat.rearrange("p (i j) -> p i j", i=V)
        fwt = sb.tile([B, V, V], DT)
        for k in range(V):
            src = dm3 if k == 0 else M3
            nc.vector.tensor_tensor(
                out=fwt[:],
                in0=src[:, :, k : k + 1].broadcast_to([B, V, V]),
                in1=src[:, k : k + 1, :].broadcast_to([B, V, V]),
                op=A.max,
            )
            nc.vector.tensor_tensor(out=M3[:], in0=src[:], in1=fwt[:], op=A.min)

        # ---- MST mask + masked upper-tri values ----
        mk = sb.tile([B, VV], DT)
        nc.vector.tensor_tensor(out=mk[:], in0=M[:], in1=dmat[:], op=A.is_ge)
        val = sb.tile([B, VV], DT)
        nc.vector.tensor_tensor(out=val[:], in0=mk[:], in1=dut[:], op=A.mult)

        # ---- extract 24 MST weights: 3 rounds of top-8 + match_replace ----
        deaths = sb.tile([B, NT], DT)
        mr1 = sb.tile([B, VV], DT)
        mr2 = sb.tile([B, VV], DT)
        nc.vector.max(deaths[:, 0:8], val[:])
        nc.vector.match_replace(mr1[:], deaths[:, 0:8], val[:], 0.0)
        nc.vector.max(deaths[:, 8:16], mr1[:])
        nc.vector.match_replace(mr2[:], deaths[:, 8:16], mr1[:], 0.0)
        nc.vector.max(deaths[:, 16:24], mr2[:])

        # ---- pre-AR: w' = deaths - 1e-6, w'^2, ONE PE transpose to [64, B]
        # (w'^2 in cols 0:24, w' in cols 32:56 -- partition-aligned blocks) ----
        w12 = sb.tile([B, K2], DT)
        w2 = w12[:, 0:NT]
        w1 = w12[:, 32 : 32 + NT]
        nc.gpsimd.tensor_scalar_add(w1, deaths[:], -GMIN)
        nc.gpsimd.tensor_tensor(out=w2, in0=w1, in1=w1, op=A.mult)
        ps_wT = psum.tile([K2, B], DT)
        with tc.tile_wait_until(0.0705):
            nc.tensor.matmul(out=ps_wT[:], lhsT=w12[:], rhs=id64[:],
                             start=True, stop=True)

        # ---- post-AR: scale lhsT rows (Scalar: activation-with-AP-scale
        # keeps the Vector queue free), then chunked PE/exp/reduce ----
        with tc.tile_wait_until(0.073):
            nc.vector.tensor_scalar_mul(LT[0:NT, :], ps_wT[0:NT, :],
                                        iv2[0:NT, 1:2])
            nc.vector.tensor_scalar_mul(LT[32 : 32 + NT, :],
                                        ps_wT[32 : 32 + NT, :], iv2[0:NT, 0:1])
        # PE slices (e 0:32): one PSUM bank (512 floats) per chunk -- a
        # matmul output must not cross a 2KB bank boundary
        EP = 14          # e-channels per PE chunk
        EV0, EV1 = 2 * EP, 50
        ps_s = psum.tile([B, 2, 512], DT)
        fexp = sb.tile([B, E, NT], DT)
        S = sb.tile([B, E], DT)
        # dn = w' * inv on Scalar (activation scale can be a [p,1] AP)
        dn = sb.tile([B, NT], DT)
        with tc.tile_wait_until(0.073):
            nc.vector.tensor_scalar_mul(dn[:], w1, iv2[:, 0:1])
        for ch in range(2):
            e0, e1 = ch * EP, (ch + 1) * EP
            sview = ps_s[:, ch, 0 : EP * NT].rearrange("p (e q) -> p e q", e=EP)
            nc.tensor.matmul(out=sview, lhsT=LT[:],
                             rhs=rhs[:, e0:e1, :], start=True, stop=True)
            nc.scalar.activation(fexp[:, e0:e1, :], sview, ACT.Exp,
                                 bias=0.0, scale=-1.0)
            nc.vector.tensor_reduce(out=S[:, e0:e1], in_=fexp[:, e0:e1, :],
                                    axis=mybir.AxisListType.X, op=A.add)
        # direct slices: u*(dn - c2)^2 on DVE (fresh tiles: in-place strided
        # ops run ~2x slower) and GpSimd
        tD = sb.tile([B, EV1 - EV0, NT], DT)
        tD2 = sb.tile([B, EV1 - EV0, NT], DT)
        nc.vector.tensor_tensor(
            out=tD[:],
            in0=dn.unsqueeze(1).broadcast_to([B, EV1 - EV0, NT]),
            in1=cu2[:, 0, EV0:EV1].unsqueeze(-1).broadcast_to([B, EV1 - EV0, NT]),
            op=A.subtract,
        )
        nc.vector.tensor_tensor(out=tD2[:], in0=tD[:], in1=tD[:], op=A.mult)
        nc.vector.tensor_tensor(
            out=tD[:], in0=tD2[:],
            in1=cu2[:, 1, EV0:EV1].unsqueeze(-1).broadcast_to([B, EV1 - EV0, NT]),
            op=A.mult,
        )
        nc.scalar.activation(fexp[:, EV0:EV1, :], tD[:], ACT.Exp,
                             bias=0.0, scale=-1.0)
        nc.vector.tensor_reduce(out=S[:, EV0:EV1], in_=fexp[:, EV0:EV1, :],
                                axis=mybir.AxisListType.X, op=A.add)
        tG = sb.tile([B, E - EV1, NT], DT)
        tG2 = sb.tile([B, E - EV1, NT], DT)
        nc.gpsimd.tensor_tensor(
            out=tG[:],
            in0=dn.unsqueeze(1).broadcast_to([B, E - EV1, NT]),
            in1=cu2[:, 0, EV1:E].unsqueeze(-1).broadcast_to([B, E - EV1, NT]),
            op=A.subtract,
        )
        nc.gpsimd.tensor_tensor(out=tG2[:], in0=tG[:], in1=tG[:], op=A.mult)
        nc.gpsimd.tensor_tensor(
            out=tG[:], in0=tG2[:],
            in1=cu2[:, 1, EV1:E].unsqueeze(-1).broadcast_to([B, E - EV1, NT]),
            op=A.mult,
        )
        nc.scalar.activation(fexp[:, EV1:E, :], tG[:], ACT.Exp,
                             bias=0.0, scale=-1.0)
        nc.vector.tensor_reduce(out=S[:, EV1:E], in_=fexp[:, EV1:E, :],
                                axis=mybir.AxisListType.X, op=A.add)
        outt = sb.tile([B, E], DT)
        nc.vector.tensor_tensor(out=outt[:], in0=S[:], in1=Ab[:], op=A.mult)
        nc.sync.dma_start(out_d[:], outt[:])

    _split_excess_waits(nc)
    return nc


_CACHE = {}


def _consts():
    # pair matrix: adds partition rows b and b+64 (the two T-halves) and
    # applies the 1/T mean scale
    pairmat = np.zeros((128, B), dtype=np.float32)
    for p in range(128):
        pairmat[p, p % B] = 1.0 / T
    ut = np.broadcast_to(
        np.triu(np.ones((V, V), dtype=np.float32), k=1).reshape(1, VV), (B, VV)
    ).copy()
    return pairmat, np.ascontiguousarray(ut), np.eye(B, dtype=np.float32)


def _param_consts(centres, sharpness):
    """Parameter-only preprocessing (host): the structure-element rhs
    [K2, E*NT] for the K2-deep PE contraction, and the pre-broadcast
    Ab = exp(-(s1*c1)^2) factor.

      s[b,(e,p)] = inv^2*w2[b,p]*u_e - 2*inv*w1[b,p]*(u*c2)_e + (u*c2^2)_e
      rows 0:24  = delta(p==k) * u_e        (paired with LT w'^2 block)
      rows 32:56 = delta(p==k) * (-2*u*c2)_e (paired with LT w' block)
      row  56    = (u*c2^2)_e                (paired with LT ones row)
    """
    u = (sharpness[:, 1] ** 2).astype(np.float32)
    c2 = centres[:, 1].astype(np.float32)
    rhs = np.zeros((K2, E, NT), dtype=np.float32)
    for k in range(NT):
        rhs[k, :, k] = u
        rhs[32 + k, :, k] = -2.0 * u * c2
    rhs[ROW_Q] = (u * c2 * c2)[:, None]
    ab = np.exp(-((sharpness[:, 0] * centres[:, 0]) ** 2)).astype(np.float32)
    ab_b = np.broadcast_to(ab[None, :], (B, E)).copy()
    cu = np.concatenate([c2, u]).astype(np.float32).reshape(1, 2 * E)
    return np.ascontiguousarray(rhs.reshape(K2, E * NT)), ab_b, np.ascontiguousarray(cu)


def _get_program():
    if "nc" not in _CACHE:
        _CACHE["nc"] = _build_program()
    return _CACHE["nc"]


def _run(x, centres, sharpness, **run_kwargs):
    nc = _get_program()
    xf = np.ascontiguousarray(x.reshape(-1, C, T, V)).astype(np.float32, copy=False)
    n_total = xf.shape[0]
    assert n_total == N_CORES * B, xf.shape
    pairmat, ut, id64 = _consts()
    rhs, ab, cu = _param_consts(np.asarray(centres), np.asarray(sharpness))
    in_maps = [
        {
            "x": np.ascontiguousarray(xf[i * B : (i + 1) * B]),
            "pm": pairmat,
            "ut": ut,
            "id64": id64,
            "rhs": rhs,
            "ab": ab,
            "cu": cu,
        }
        for i in range(N_CORES)
    ]
    res = run_bass_kernel_spmd(nc, in_maps, list(range(N_CORES)), **run_kwargs)
    out = np.concatenate([res.results[i]["out"] for i in range(N_CORES)], axis=0)
    return out, res


def kernel(x, centres, sharpness):
    out, _ = _run(np.asarray(x), np.asarray(centres), np.asarray(sharpness))
    return out


# revision 23
# speedup vs baseline: 1.1811x; 1.1811x over previous
"""TopoEncoder Trainium2 kernel (8 NeuronCores, data-parallel over batch).

Pipeline per core (64 samples):
  1. DMA x-shard (two HWDGE queues, 3.2KB descriptors), mean over T
     (DVE+GpSimd add-tree, PE pair-matrix fold)
  2. pairwise channel-L2 distance matrix d [64,25,25]
  3. local max -> PE transpose -> AllReduce(max) across the 8 cores,
     triggered immediately. Its latency is dominated by the CC-stream
     entry barrier (~54us) + ~23us fixed op cost; everything through the
     top-24 extraction hides under it. Global min is NOT communicated:
     the distance-matrix diagonal is exactly sqrt(1e-12) = 1e-6 on every
     sample, so global min == 1e-6 always.
  4. Floyd-Warshall min-max closure M (25 steps) -> MST mask = (M >= d)
     (0-dim persistence deaths = MST edge weight multiset; downstream
      structure-element sum is permutation-invariant, so order is free)
  5. top-24 extraction of masked upper-tri values (max8 + match_replace)
  6. structure-element layer via a single K=49 PE contraction:
       s[b,(e,p)] = inv^2*u_e*w'^2[b,p] - 2*inv*u_e*c2_e*w'[b,p] + u_e*c2_e^2
                  = u_e * ((w'[b,p] * inv) - c2_e)^2,   w' = deaths - 1e-6
     The death-index selector (delta mask) and parameter rows are baked
     into a [49, E*NT] rhs during the DMA-wait head; post-AllReduce work
     is only: reciprocal, two [24,64] row scales, 4 chunked PE matmuls,
     exp (Scalar), and per-e reduce (DVE) -> ~5us tail.

All AR-dependent prep runs on GpSimd/PE/Scalar or is forced late via
tile_wait_until so the Vector queue never blocks on the collective.

All constant broadcasts across partitions are built with K=1 PE matmuls
(ones[1,B] (x) row) — partition-broadcast DMAs on SWDGE are ~7x slower
per packet and starve the big x transfer.
"""

from contextlib import ExitStack

import numpy as np

import bass_rust
import concourse.bass as bass
import concourse.tile as tile
from concourse import mybir
from concourse.bass_utils import run_bass_kernel_spmd

N_CORES = 8
B = 64          # samples per core
C, T, V, E = 3, 128, 25, 64
VV = V * V
NT = V - 1      # deaths per sample (24)
K2 = 64          # PE contraction depth for the structure-element layer
ROW_Q = 56       # rhs row carrying the constant (u*c2^2) term
DT = mybir.dt.float32
GMIN = 1e-6     # exact global min of d: the diagonal is sqrt(1e-12)
ECH = 16        # e-channels per tail chunk (PE/exp/reduce pipeline)


def _split_excess_waits(nc, cap=1):
    """The walrus build in this env rejects instructions carrying more than
    ~2 semaphore-wait commands. Move excess waits onto same-engine NOPs
    inserted immediately before the offending instruction."""
    n_split = 0
    for bb in nc.main_func.blocks:
        insts = bb.instructions
        i = 0
        while i < len(insts):
            ins = insts[i]
            si = ins.sync_info
            waits = list(si.on_wait) if si and si.on_wait else []
            if len(waits) > cap:
                extra, keep = waits[:-cap], waits[-cap:]
                ins.sync_info = mybir.SyncInfo(
                    on_wait=keep, on_update=list(si.on_update or [])
                )
                for j, w in enumerate(extra):
                    nop = bass_rust.InstNoOp(
                        name=f"I-wsplit-{n_split}-{j}",
                        engine=ins.engine,
                        sync_info=mybir.SyncInfo(on_wait=[w], on_update=[]),
                    )
                    insts.insert(i, nop)
                    i += 1
                n_split += 1
            i += 1
    return n_split


def _build_program():
    A = mybir.AluOpType
    ACT = mybir.ActivationFunctionType
    nc = bass.Bass("TRN2", debug=False, num_devices=N_CORES)

    x_in = nc.dram_tensor("x", [B, C, T, V], DT, kind="ExternalInput").ap()
    pm_in = nc.dram_tensor("pm", [128, B], DT, kind="ExternalInput").ap()
    ut_in = nc.dram_tensor("ut", [B, VV], DT, kind="ExternalInput").ap()
    id_in = nc.dram_tensor("id64", [B, B], DT, kind="ExternalInput").ap()
    rhs_in = nc.dram_tensor("rhs", [K2, E * NT], DT, kind="ExternalInput").ap()
    ab_in = nc.dram_tensor("ab", [B, E], DT, kind="ExternalInput").ap()
    cu_in = nc.dram_tensor("cu", [1, 2 * E], DT, kind="ExternalInput").ap()
    out_d = nc.dram_tensor("out", [B, E], DT, kind="ExternalOutput").ap()

    with tile.TileContext(nc, num_cores=N_CORES) as tc, ExitStack() as ctx:
        sb = ctx.enter_context(tc.tile_pool(name="sb", bufs=1))
        psum = ctx.enter_context(tc.tile_pool(name="psum", bufs=1, space="PSUM"))
        dram = ctx.enter_context(tc.tile_pool(name="dram", bufs=1, space="DRAM"))

        # ---- x DMA first: partition p = t2*64 + b, free = (c, t64, v) ----
        # two t64-half tiles so the add-tree overlaps the second half's DMA;
        # both HWDGE queues used; 3.2KB contiguous descriptors
        xa = sb.tile([128, C, T // 4, V], DT)
        xb = sb.tile([128, C, T // 4, V], DT)
        nc.sync.dma_start(xa[0:B], x_in[:, :, 0:32, :])
        nc.scalar.dma_start(xa[96:128], x_in[32:64, :, 64:96, :])
        nc.sync.dma_start(xa[B:96], x_in[0:32, :, 64:96, :])
        nc.sync.dma_start(xb[0:54], x_in[0:54, :, 32:64, :])
        nc.scalar.dma_start(xb[54:B], x_in[54:B, :, 32:64, :])
        nc.scalar.dma_start(xb[B:128], x_in[:, :, 96:128, :])

        # ---- small constant loads (HWDGE, few descriptors) ----
        pm_t = sb.tile([128, B], DT)
        nc.sync.dma_start(pm_t[:], pm_in[:])
        id64 = sb.tile([B, B], DT)
        nc.sync.dma_start(id64[:], id_in[:])
        rhs = sb.tile([K2, E, NT], DT)
        nc.sync.dma_start(rhs[:], rhs_in[:])
        Ab = sb.tile([B, E], DT)
        nc.scalar.dma_start(Ab[:], ab_in[:])
        utb = sb.tile([B, VV], DT)
        nc.scalar.dma_start(utb[:], ut_in[:])
        ones1 = sb.tile([1, B], DT)
        nc.vector.memset(ones1[:], 1.0)
        eps = sb.tile([128, 1], DT)
        nc.vector.memset(eps[:], 1e-12)

        # [c2 | u] broadcast rows for the direct DVE/GpSimd SEL slices
        cu_in_row = sb.tile([1, 2, E], DT)
        nc.scalar.dma_start(cu_in_row[:], cu_in[:])
        bc2 = psum.tile([B, 2, E], DT)
        nc.tensor.matmul(out=bc2[:], lhsT=ones1[:], rhs=cu_in_row[:],
                         start=True, stop=True)
        cu2 = sb.tile([B, 2, E], DT)
        nc.vector.tensor_copy(cu2[:], bc2[:])

        # structure-element lhsT rows: [w'^2.T * inv^2; w'.T * inv; ones]
        # (memset must start at partition 0: set all rows, the first 48 are
        # overwritten by the post-AR scales)
        LT = sb.tile([K2, B], DT)
        nc.vector.memset(LT[:], 1.0)

        # PWP management: load the Sqrt table early (dummy, no deps) so the
        # d-matrix sqrt pays no mid-chain table switch; the Exp table is
        # loaded after dmat (dummy depends on it) so the tail exps are clean
        dumm = sb.tile([1, 2], DT)
        nc.vector.memset(dumm[:], 0.0)
        nc.scalar.sqrt(dumm[:], dumm[:])

        # ---- mean over T: in-place add trees (DVE: c0-c1, GpSimd: c2),
        # then PE pair-matrix fold ----
        for xh in (xa, xb):
            for w in (16, 8, 4, 2, 1):
                nc.vector.tensor_tensor(
                    out=xh[:, 0:2, 0:w, :],
                    in0=xh[:, 0:2, 0:w, :],
                    in1=xh[:, 0:2, w : 2 * w, :],
                    op=A.add,
                )
                nc.gpsimd.tensor_tensor(
                    out=xh[:, 2, 0:w, :],
                    in0=xh[:, 2, 0:w, :],
                    in1=xh[:, 2, w : 2 * w, :],
                    op=A.add,
                )
        nc.vector.tensor_tensor(
            out=xa[:, 0:2, 0:1, :], in0=xa[:, 0:2, 0:1, :], in1=xb[:, 0:2, 0:1, :],
            op=A.add,
        )
        nc.gpsimd.tensor_tensor(
            out=xa[:, 2, 0:1, :], in0=xa[:, 2, 0:1, :], in1=xb[:, 2, 0:1, :],
            op=A.add,
        )
        ps_xm = psum.tile([B, C, V], DT)
        nc.tensor.matmul(out=ps_xm[:], lhsT=pm_t[:], rhs=xa[:, :, 0, :],
                         start=True, stop=True)
        xm = sb.tile([B, C, V], DT)
        nc.vector.tensor_copy(xm[:], ps_xm[:])

        # ---- distance matrix ----
        df = sb.tile([B, C, V, V], DT)
        xmb_i = xm.unsqueeze(-1).broadcast_to([B, C, V, V])
        xmb_j = xm.unsqueeze(2).broadcast_to([B, C, V, V])
        nc.vector.tensor_tensor(
            out=df[:, 0:2], in0=xmb_i[:, 0:2], in1=xmb_j[:, 0:2], op=A.subtract
        )
        nc.gpsimd.tensor_tensor(
            out=df[:, 2], in0=xmb_i[:, 2], in1=xmb_j[:, 2], op=A.subtract
        )
        nc.scalar.square(df[:, 0:2], df[:, 0:2])
        nc.vector.tensor_tensor(out=df[:, 2], in0=df[:, 2], in1=df[:, 2], op=A.mult)
        d2 = sb.tile([B, VV], DT)
        d23 = d2.rearrange("p (i j) -> p i j", i=V)
        nc.vector.tensor_tensor(out=d23[:], in0=df[:, 0], in1=df[:, 1], op=A.add)
        nc.vector.tensor_tensor(out=d23[:], in0=d23[:], in1=df[:, 2], op=A.add)
        dmat = sb.tile([B, VV], DT)
        nc.scalar.activation(dmat[:], d2[:], ACT.Sqrt, bias=eps[0:B, 0:1], scale=1.0)

        # ---- local max from d^2 (sqrt commutes with max; the [B,VV] reduce
        # overlaps the Scalar-engine sqrt) -> PE transpose -> AllReduce ----
        lmax = sb.tile([B, 1], DT)
        nc.vector.tensor_reduce(out=lmax[:], in_=d2[:],
                                axis=mybir.AxisListType.X, op=A.max)
        lmaxT = psum.tile([1, B], DT)
        nc.tensor.matmul(out=lmaxT[:], lhsT=lmax[:], rhs=id64[:], start=True, stop=True)
        gmr2 = sb.tile([1, 1], DT)
        nc.vector.tensor_reduce(out=gmr2[:], in_=lmaxT[:],
                                axis=mybir.AxisListType.X, op=A.max)
        gmr = sb.tile([1, 1], DT)
        nc.scalar.activation(gmr[:], gmr2[:], ACT.Sqrt, bias=eps[0:1, 0:1], scale=1.0)
        # Exp-table preload pinned AFTER the last Sqrt (reads gmr) so the
        # tail exps pay no PWP reload
        nc.scalar.activation(dumm[:], gmr[0:1, 0:1].broadcast_to([1, 2]), ACT.Exp,
                             bias=0.0, scale=-1.0)
        cin = dram.tile([1, 1], DT)
        cout = dram.tile([1, 1], DT)
        nc.scalar.dma_start(cin[:], gmr[:])
        nc.gpsimd.collective_compute(
            "AllReduce", A.max, replica_groups=[list(range(N_CORES))],
            ins=[cin.opt()], outs=[cout.opt()],
        )
        # AR-dependent scalar prep: rng and 1/rng on the [1,1] scalar FIRST,
        # then one PE matmul broadcasts [inv, inv^2] to all partitions
        gsb = sb.tile([1, 1], DT)
        rng0 = sb.tile([1, 1], DT)
        ivrow = sb.tile([1, 2], DT)
        ivp = psum.tile([B, 2], DT)
        iv2 = sb.tile([B, 2], DT)
        with tc.tile_wait_until(0.068):
            nc.scalar.dma_start(gsb[:], cout[:])
        with tc.tile_wait_until(0.0695):
            nc.vector.tensor_scalar_add(rng0[:], gsb[:], -GMIN)
            nc.vector.reciprocal(ivrow[:, 0:1], rng0[:])
            nc.vector.tensor_tensor(out=ivrow[:, 1:2], in0=ivrow[:, 0:1],
                                    in1=ivrow[:, 0:1], op=A.mult)
            nc.tensor.matmul(out=ivp[:], lhsT=ones1[:], rhs=ivrow[:],
                             start=True, stop=True)
            nc.vector.tensor_copy(iv2[:], ivp[:])
        invb = iv2[:, 0:1]
        inv2b = iv2[:, 1:2]

        # ---- premasked values on GpSimd (idle during FW) ----
        dut = sb.tile([B, VV], DT)
        nc.gpsimd.tensor_tensor(out=dut[:], in0=dmat[:], in1=utb[:], op=A.mult)

        # ---- Floyd-Warshall min-max closure (in place) ----
        M = sb.tile([B, VV], DT)
        M3 = M.rearrange("p (i j) -> p i j", i=V)
        dm3 = dmat.rearrange("p (i j) -> p i j", i=V)
        fwt = sb.tile([B, V, V], DT)
        for k in range(V):
            src = dm3 if k == 0 else M3
            nc.vector.tensor_tensor(
                out=fwt[:],
                in0=src[:, :, k : k + 1].broadcast_to([B, V, V]),
                in1=src[:, k : k + 1, :].broadcast_to([B, V, V]),
                op=A.max,
            )
            nc.vector.tensor_tensor(out=M3[:], in0=src[:], in1=fwt[:], op=A.min)

        # ---- MST mask + masked upper-tri values ----
        mk = sb.tile([B, VV], DT)
        nc.vector.tensor_tensor(out=mk[:], in0=M[:], in1=dmat[:], op=A.is_ge)
        val = sb.tile([B, VV], DT)
        nc.vector.tensor_tensor(out=val[:], in0=mk[:], in1=dut[:], op=A.mult)

        # ---- extract 24 MST weights: 3 rounds of top-8 + match_replace ----
        deaths = sb.tile([B, NT], DT)
        mr1 = sb.tile([B, VV], DT)
        mr2 = sb.tile([B, VV], DT)
        nc.vector.max(deaths[:, 0:8], val[:])
        nc.vector.match_replace(mr1[:], deaths[:, 0:8], val[:], 0.0)
        nc.vector.max(deaths[:, 8:16], mr1[:])
        nc.vector.match_replace(mr2[:], deaths[:, 8:16], mr1[:], 0.0)
        nc.vector.max(deaths[:, 16:24], mr2[:])

        # ---- pre-AR: w' = deaths - 1e-6, w'^2, ONE PE transpose to [64, B]
        # (w'^2 in cols 0:24, w' in cols 32:56 -- partition-aligned blocks) ----
        w12 = sb.tile([B, K2], DT)
        w2 = w12[:, 0:NT]
        w1 = w12[:, 32 : 32 + NT]
        nc.gpsimd.tensor_scalar_add(w1, deaths[:], -GMIN)
        nc.gpsimd.tensor_tensor(out=w2, in0=w1, in1=w1, op=A.mult)
        ps_wT = psum.tile([K2, B], DT)
        with tc.tile_wait_until(0.0705):
            nc.tensor.matmul(out=ps_wT[:], lhsT=w12[:], rhs=id64[:],
                             start=True, stop=True)

        # ---- post-AR: scale lhsT rows (Scalar: activation-with-AP-scale
        # keeps the Vector queue free), then chunked PE/exp/reduce ----
        with tc.tile_wait_until(0.073):
            nc.vector.tensor_scalar_mul(LT[0:NT, :], ps_wT[0:NT, :],
                                        iv2[0:NT, 1:2])
            nc.vector.tensor_scalar_mul(LT[32 : 32 + NT, :],
                                        ps_wT[32 : 32 + NT, :], iv2[0:NT, 0:1])
        # PE slices (e 0:32): one PSUM bank (512 floats) per chunk -- a
        # matmul output must not cross a 2KB bank boundary
        EP = 14          # e-channels per PE chunk
        EV0, EV1 = 2 * EP, 50
        ps_s = psum.tile([B, 2, 512], DT)
        fexp = sb.tile([B, E, NT], DT)
        S = sb.tile([B, E], DT)
        # dn = w' * inv on Scalar (activation scale can be a [p,1] AP)
        dn = sb.tile([B, NT], DT)
        with tc.tile_wait_until(0.073):
            nc.vector.tensor_scalar_mul(dn[:], w1, iv2[:, 0:1])
        for ch in range(2):
            e0, e1 = ch * EP, (ch + 1) * EP
            sview = ps_s[:, ch, 0 : EP * NT].rearrange("p (e q) -> p e q", e=EP)
            nc.tensor.matmul(out=sview, lhsT=LT[:],
                             rhs=rhs[:, e0:e1, :], start=True, stop=True)
            nc.scalar.activation(fexp[:, e0:e1, :], sview, ACT.Exp,
                                 bias=0.0, scale=-1.0)
            nc.vector.tensor_reduce(out=S[:, e0:e1], in_=fexp[:, e0:e1, :],
                                    axis=mybir.AxisListType.X, op=A.add)
        # direct slices: u*(dn - c2)^2 on DVE (fresh tiles: in-place strided
        # ops run ~2x slower) and GpSimd
        tD = sb.tile([B, EV1 - EV0, NT], DT)
        tD2 = sb.tile([B, EV1 - EV0, NT], DT)
        nc.vector.tensor_tensor(
            out=tD[:],
            in0=dn.unsqueeze(1).broadcast_to([B, EV1 - EV0, NT]),
            in1=cu2[:, 0, EV0:EV1].unsqueeze(-1).broadcast_to([B, EV1 - EV0, NT]),
            op=A.subtract,
        )
        nc.vector.tensor_tensor(out=tD2[:], in0=tD[:], in1=tD[:], op=A.mult)
        nc.vector.tensor_tensor(
            out=tD[:], in0=tD2[:],
            in1=cu2[:, 1, EV0:EV1].unsqueeze(-1).broadcast_to([B, EV1 - EV0, NT]),
            op=A.mult,
        )
        nc.scalar.activation(fexp[:, EV0:EV1, :], tD[:], ACT.Exp,
                             bias=0.0, scale=-1.0)
        nc.vector.tensor_reduce(out=S[:, EV0:EV1], in_=fexp[:, EV0:EV1, :],
                                axis=mybir.AxisListType.X, op=A.add)
        tG = sb.tile([B, E - EV1, NT], DT)
        tG2 = sb.tile([B, E - EV1, NT], DT)
        nc.gpsimd.tensor_tensor(
            out=tG[:],
            in0=dn.unsqueeze(1).broadcast_to([B, E - EV1, NT]),
            in1=cu2[:, 0, EV1:E].unsqueeze(-1).broadcast_to([B, E - EV1, NT]),
            op=A.subtract,
        )
        nc.gpsimd.tensor_tensor(out=tG2[:], in0=tG[:], in1=tG[:], op=A.mult)
        nc.gpsimd.tensor_tensor(
            out=tG[:], in0=tG2[:],
            in1=cu2[:, 1, EV1:E].unsqueeze(-1).broadcast_to([B, E - EV1, NT]),
            op=A.mult,
        )
        nc.scalar.activation(fexp[:, EV1:E, :], tG[:], ACT.Exp,
                             bias=0.0, scale=-1.0)
        nc.vector.tensor_reduce(out=S[:, EV1:E], in_=fexp[:, EV1:E, :],
                                axis=mybir.AxisListType.X, op=A.add)
        outt = sb.tile([B, E], DT)
        nc.vector.tensor_tensor(out=outt[:], in0=S[:], in1=Ab[:], op=A.mult)
        nc.sync.dma_start(out_d[:], outt[:])

    _split_excess_waits(nc)
    return nc


_CACHE = {}


def _consts():
    # pair matrix: adds partition rows b and b+64 (the two T-halves) and
    # applies the 1/T mean scale
    pairmat = np.zeros((128, B), dtype=np.float32)
    for p in range(128):
        pairmat[p, p % B] = 1.0 / T
    ut = np.broadcast_to(
        np.triu(np.ones((V, V), dtype=np.float32), k=1).reshape(1, VV), (B, VV)
    ).copy()
    return pairmat, np.ascontiguousarray(ut), np.eye(B, dtype=np.float32)


def _param_consts(centres, sharpness):
    """Parameter-only preprocessing (host): the structure-element rhs
    [K2, E*NT] for the K2-deep PE contraction, and the pre-broadcast
    Ab = exp(-(s1*c1)^2) factor.

      s[b,(e,p)] = inv^2*w2[b,p]*u_e - 2*inv*w1[b,p]*(u*c2)_e + (u*c2^2)_e
      rows 0:24  = delta(p==k) * u_e        (paired with LT w'^2 block)
      rows 32:56 = delta(p==k) * (-2*u*c2)_e (paired with LT w' block)
      row  56    = (u*c2^2)_e                (paired with LT ones row)
    """
    u = (sharpness[:, 1] ** 2).astype(np.float32)
    c2 = centres[:, 1].astype(np.float32)
    rhs = np.zeros((K2, E, NT), dtype=np.float32)
    for k in range(NT):
        rhs[k, :, k] = u
        rhs[32 + k, :, k] = -2.0 * u * c2
    rhs[ROW_Q] = (u * c2 * c2)[:, None]
    ab = np.exp(-((sharpness[:, 0] * centres[:, 0]) ** 2)).astype(np.float32)
    ab_b = np.broadcast_to(ab[None, :], (B, E)).copy()
    cu = np.concatenate([c2, u]).astype(np.float32).reshape(1, 2 * E)
    return np.ascontiguousarray(rhs.reshape(K2, E * NT)), ab_b, np.ascontiguousarray(cu)


def _get_program():
    if "nc" not in _CACHE:
        _CACHE["nc"] = _build_program()
    return _CACHE["nc"]


def _run(x, centres, sharpness, **run_kwargs):
    nc = _get_program()
    xf = np.ascontiguousarray(x.reshape(-1, C, T, V)).astype(np.float32, copy=False)
    n_total = xf.shape[0]
    assert n_total == N_CORES * B, xf.shape
    pairmat, ut, id64 = _consts()
    rhs, ab, cu = _param_consts(np.asarray(centres), np.asarray(sharpness))
    in_maps = [
        {
            "x": np.ascontiguousarray(xf[i * B : (i + 1) * B]),
            "pm": pairmat,
            "ut": ut,
            "id64": id64,
            "rhs": rhs,
            "ab": ab,
            "cu": cu,
        }
        for i in range(N_CORES)
    ]
    res = run_bass_kernel_spmd(nc, in_maps, list(range(N_CORES)), **run_kwargs)
    out = np.concatenate([res.results[i]["out"] for i in range(N_CORES)], axis=0)
    return out, res


def kernel(x, centres, sharpness):
    out, _ = _run(np.asarray(x), np.asarray(centres), np.asarray(sharpness))
    return out


# revision 24
# speedup vs baseline: 1.2492x; 1.0576x over previous
"""TopoEncoder Trainium2 kernel (8 NeuronCores, data-parallel over batch).

Pipeline per core (64 samples):
  1. DMA x-shard (two HWDGE queues, 3.2KB descriptors), mean over T
     (DVE+GpSimd add-tree, PE pair-matrix fold)
  2. pairwise channel-L2 distance matrix d [64,25,25]
  3. local max -> PE transpose -> AllReduce(max) across the 8 cores,
     triggered immediately. Its latency is dominated by the CC-stream
     entry barrier (~54us) + ~23us fixed op cost; everything through the
     top-24 extraction hides under it. Global min is NOT communicated:
     the distance-matrix diagonal is exactly sqrt(1e-12) = 1e-6 on every
     sample, so global min == 1e-6 always.
  4. Floyd-Warshall min-max closure M (25 steps) -> MST mask = (M >= d)
     (0-dim persistence deaths = MST edge weight multiset; downstream
      structure-element sum is permutation-invariant, so order is free)
  5. top-24 extraction of masked upper-tri values (max8 + match_replace)
  6. structure-element layer via a single K=49 PE contraction:
       s[b,(e,p)] = inv^2*u_e*w'^2[b,p] - 2*inv*u_e*c2_e*w'[b,p] + u_e*c2_e^2
                  = u_e * ((w'[b,p] * inv) - c2_e)^2,   w' = deaths - 1e-6
     The death-index selector (delta mask) and parameter rows are baked
     into a [49, E*NT] rhs during the DMA-wait head; post-AllReduce work
     is only: reciprocal, two [24,64] row scales, 4 chunked PE matmuls,
     exp (Scalar), and per-e reduce (DVE) -> ~5us tail.

All AR-dependent prep runs on GpSimd/PE/Scalar or is forced late via
tile_wait_until so the Vector queue never blocks on the collective.

All constant broadcasts across partitions are built with K=1 PE matmuls
(ones[1,B] (x) row) — partition-broadcast DMAs on SWDGE are ~7x slower
per packet and starve the big x transfer.
"""

from contextlib import ExitStack

import numpy as np

import bass_rust
import concourse.bass as bass
import concourse.tile as tile
from concourse import mybir
from concourse.bass_utils import run_bass_kernel_spmd

N_CORES = 8
B = 64          # samples per core
C, T, V, E = 3, 128, 25, 64
VV = V * V
NT = V - 1      # deaths per sample (24)
K2 = 64          # PE contraction depth for the structure-element layer
ROW_Q = 56       # rhs row carrying the constant (u*c2^2) term
DT = mybir.dt.float32
GMIN = 1e-6     # exact global min of d: the diagonal is sqrt(1e-12)
ECH = 16        # e-channels per tail chunk (PE/exp/reduce pipeline)


def _split_excess_waits(nc, cap=1):
    """The walrus build in this env rejects instructions carrying more than
    ~2 semaphore-wait commands. Move excess waits onto same-engine NOPs
    inserted immediately before the offending instruction."""
    n_split = 0
    for bb in nc.main_func.blocks:
        insts = bb.instructions
        i = 0
        while i < len(insts):
            ins = insts[i]
            si = ins.sync_info
            waits = list(si.on_wait) if si and si.on_wait else []
            if len(waits) > cap:
                extra, keep = waits[:-cap], waits[-cap:]
                ins.sync_info = mybir.SyncInfo(
                    on_wait=keep, on_update=list(si.on_update or [])
                )
                for j, w in enumerate(extra):
                    nop = bass_rust.InstNoOp(
                        name=f"I-wsplit-{n_split}-{j}",
                        engine=ins.engine,
                        sync_info=mybir.SyncInfo(on_wait=[w], on_update=[]),
                    )
                    insts.insert(i, nop)
                    i += 1
                n_split += 1
            i += 1
    return n_split


def _build_program():
    A = mybir.AluOpType
    ACT = mybir.ActivationFunctionType
    nc = bass.Bass("TRN2", debug=False, num_devices=N_CORES)

    x_in = nc.dram_tensor("x", [B, C, T, V], DT, kind="ExternalInput").ap()
    pm_in = nc.dram_tensor("pm", [128, B], DT, kind="ExternalInput").ap()
    ut_in = nc.dram_tensor("ut", [B, VV], DT, kind="ExternalInput").ap()
    id_in = nc.dram_tensor("id64", [B, B], DT, kind="ExternalInput").ap()
    rhs_in = nc.dram_tensor("rhs", [K2, E * NT], DT, kind="ExternalInput").ap()
    ab_in = nc.dram_tensor("ab", [B, E], DT, kind="ExternalInput").ap()
    cu_in = nc.dram_tensor("cu", [1, 2 * E], DT, kind="ExternalInput").ap()
    out_d = nc.dram_tensor("out", [B, E], DT, kind="ExternalOutput").ap()

    with tile.TileContext(nc, num_cores=N_CORES) as tc, ExitStack() as ctx:
        sb = ctx.enter_context(tc.tile_pool(name="sb", bufs=1))
        psum = ctx.enter_context(tc.tile_pool(name="psum", bufs=1, space="PSUM"))
        dram = ctx.enter_context(tc.tile_pool(name="dram", bufs=1, space="DRAM"))

        # ---- x DMA first: partition p = t2*64 + b, free = (c, t64, v) ----
        # two t64-half tiles so the add-tree overlaps the second half's DMA;
        # both HWDGE queues used; 3.2KB contiguous descriptors
        xa = sb.tile([128, C, T // 4, V], DT)
        xb = sb.tile([128, C, T // 4, V], DT)
        nc.sync.dma_start(xa[0:B], x_in[:, :, 0:32, :])
        nc.scalar.dma_start(xa[96:128], x_in[32:64, :, 64:96, :])
        nc.sync.dma_start(xa[B:96], x_in[0:32, :, 64:96, :])
        nc.sync.dma_start(xb[0:B], x_in[:, :, 32:64, :])
        nc.scalar.dma_start(xb[B:128], x_in[:, :, 96:128, :])

        # ---- small constant loads (HWDGE, few descriptors) ----
        pm_t = sb.tile([128, B], DT)
        nc.sync.dma_start(pm_t[:], pm_in[:])
        id64 = sb.tile([B, B], DT)
        nc.sync.dma_start(id64[:], id_in[:])
        rhs = sb.tile([K2, E, NT], DT)
        nc.sync.dma_start(rhs[:], rhs_in[:])
        Ab = sb.tile([B, E], DT)
        nc.scalar.dma_start(Ab[:], ab_in[:])
        utb = sb.tile([B, VV], DT)
        nc.scalar.dma_start(utb[:], ut_in[:])
        ones1 = sb.tile([1, B], DT)
        nc.vector.memset(ones1[:], 1.0)
        eps = sb.tile([128, 1], DT)
        nc.vector.memset(eps[:], 1e-12)

        # [c2 | u] broadcast rows for the direct DVE/GpSimd SEL slices
        cu_in_row = sb.tile([1, 2, E], DT)
        nc.scalar.dma_start(cu_in_row[:], cu_in[:])
        bc2 = psum.tile([B, 2, E], DT)
        nc.tensor.matmul(out=bc2[:], lhsT=ones1[:], rhs=cu_in_row[:],
                         start=True, stop=True)
        cu2 = sb.tile([B, 2, E], DT)
        nc.vector.tensor_copy(cu2[:], bc2[:])

        # structure-element lhsT rows: [w'^2.T * inv^2; w'.T * inv; ones]
        # (memset must start at partition 0: set all rows, the first 48 are
        # overwritten by the post-AR scales)
        LT = sb.tile([K2, B], DT)
        nc.vector.memset(LT[:], 1.0)

        # PWP management: load the Sqrt table early (dummy, no deps) so the
        # d-matrix sqrt pays no mid-chain table switch; the Exp table is
        # loaded after dmat (dummy depends on it) so the tail exps are clean
        dumm = sb.tile([1, 2], DT)
        nc.vector.memset(dumm[:], 0.0)
        nc.scalar.sqrt(dumm[:], dumm[:])

        # ---- mean over T: in-place add trees (DVE: c0-c1, GpSimd: c2),
        # then PE pair-matrix fold ----
        for xh in (xa, xb):
            for w in (16, 8, 4, 2, 1):
                nc.vector.tensor_tensor(
                    out=xh[:, 0:2, 0:w, :],
                    in0=xh[:, 0:2, 0:w, :],
                    in1=xh[:, 0:2, w : 2 * w, :],
                    op=A.add,
                )
                nc.gpsimd.tensor_tensor(
                    out=xh[:, 2, 0:w, :],
                    in0=xh[:, 2, 0:w, :],
                    in1=xh[:, 2, w : 2 * w, :],
                    op=A.add,
                )
        nc.vector.tensor_tensor(
            out=xa[:, 0:2, 0:1, :], in0=xa[:, 0:2, 0:1, :], in1=xb[:, 0:2, 0:1, :],
            op=A.add,
        )
        nc.gpsimd.tensor_tensor(
            out=xa[:, 2, 0:1, :], in0=xa[:, 2, 0:1, :], in1=xb[:, 2, 0:1, :],
            op=A.add,
        )
        ps_xm = psum.tile([B, C, V], DT)
        nc.tensor.matmul(out=ps_xm[:], lhsT=pm_t[:], rhs=xa[:, :, 0, :],
                         start=True, stop=True)
        xm = sb.tile([B, C, V], DT)
        nc.vector.tensor_copy(xm[:], ps_xm[:])

        # ---- distance matrix ----
        df = sb.tile([B, C, V, V], DT)
        xmb_i = xm.unsqueeze(-1).broadcast_to([B, C, V, V])
        xmb_j = xm.unsqueeze(2).broadcast_to([B, C, V, V])
        nc.vector.tensor_tensor(
            out=df[:, 0:2], in0=xmb_i[:, 0:2], in1=xmb_j[:, 0:2], op=A.subtract
        )
        nc.gpsimd.tensor_tensor(
            out=df[:, 2], in0=xmb_i[:, 2], in1=xmb_j[:, 2], op=A.subtract
        )
        nc.scalar.square(df[:, 0:2], df[:, 0:2])
        nc.vector.tensor_tensor(out=df[:, 2], in0=df[:, 2], in1=df[:, 2], op=A.mult)
        d2 = sb.tile([B, VV], DT)
        d23 = d2.rearrange("p (i j) -> p i j", i=V)
        nc.vector.tensor_tensor(out=d23[:], in0=df[:, 0], in1=df[:, 1], op=A.add)
        nc.vector.tensor_tensor(out=d23[:], in0=d23[:], in1=df[:, 2], op=A.add)
        dmat = sb.tile([B, VV], DT)
        nc.scalar.activation(dmat[:], d2[:], ACT.Sqrt, bias=eps[0:B, 0:1], scale=1.0)
        # Exp-table preload, pinned after the sqrt via the dmat read
        nc.scalar.activation(dumm[:], dmat[0:1, 0:2], ACT.Exp, bias=0.0, scale=-1.0)

        # ---- local max from d^2 (sqrt commutes with max; the [B,VV] reduce
        # overlaps the Scalar-engine sqrt) -> PE transpose -> AllReduce ----
        lmax = sb.tile([B, 1], DT)
        nc.vector.tensor_reduce(out=lmax[:], in_=d2[:],
                                axis=mybir.AxisListType.X, op=A.max)
        lmaxT = psum.tile([1, B], DT)
        nc.tensor.matmul(out=lmaxT[:], lhsT=lmax[:], rhs=id64[:], start=True, stop=True)
        gmr2 = sb.tile([1, 1], DT)
        nc.vector.tensor_reduce(out=gmr2[:], in_=lmaxT[:],
                                axis=mybir.AxisListType.X, op=A.max)
        gmr = sb.tile([1, 1], DT)
        nc.scalar.activation(gmr[:], gmr2[:], ACT.Sqrt, bias=eps[0:1, 0:1], scale=1.0)
        cin = dram.tile([1, 1], DT)
        cout = dram.tile([1, 1], DT)
        nc.scalar.dma_start(cin[:], gmr[:])
        nc.gpsimd.collective_compute(
            "AllReduce", A.max, replica_groups=[list(range(N_CORES))],
            ins=[cin.opt()], outs=[cout.opt()],
        )
        # AR-dependent scalar prep: inv = 1/(gmax - 1e-6) on all partitions
        gsb = sb.tile([1, 1], DT)
        gbp = psum.tile([B, 1], DT)
        gb = sb.tile([B, 1], DT)
        rngb = sb.tile([B, 1], DT)
        invb = sb.tile([B, 1], DT)
        inv2b = sb.tile([B, 1], DT)
        with tc.tile_wait_until(0.068):
            nc.scalar.dma_start(gsb[:], cout[:])
            nc.tensor.matmul(out=gbp[:], lhsT=ones1[:], rhs=gsb[:],
                             start=True, stop=True)
            nc.scalar.copy(gb[:], gbp[:])
            nc.gpsimd.tensor_scalar_add(rngb[:], gb[:], -GMIN)
        with tc.tile_wait_until(0.072):
            nc.vector.reciprocal(invb[:], rngb[:])
            nc.gpsimd.tensor_tensor(out=inv2b[:], in0=invb[:], in1=invb[:],
                                    op=A.mult)

        # ---- premasked values on GpSimd (idle during FW) ----
        dut = sb.tile([B, VV], DT)
        nc.gpsimd.tensor_tensor(out=dut[:], in0=dmat[:], in1=utb[:], op=A.mult)

        # ---- Floyd-Warshall min-max closure (in place) ----
        M = sb.tile([B, VV], DT)
        M3 = M.rearrange("p (i j) -> p i j", i=V)
        dm3 = dmat.rearrange("p (i j) -> p i j", i=V)
        fwt = sb.tile([B, V, V], DT)
        for k in range(V):
            src = dm3 if k == 0 else M3
            nc.vector.tensor_tensor(
                out=fwt[:],
                in0=src[:, :, k : k + 1].broadcast_to([B, V, V]),
                in1=src[:, k : k + 1, :].broadcast_to([B, V, V]),
                op=A.max,
            )
            nc.vector.tensor_tensor(out=M3[:], in0=src[:], in1=fwt[:], op=A.min)

        # ---- MST mask + masked upper-tri values ----
        mk = sb.tile([B, VV], DT)
        nc.vector.tensor_tensor(out=mk[:], in0=M[:], in1=dmat[:], op=A.is_ge)
        val = sb.tile([B, VV], DT)
        nc.vector.tensor_tensor(out=val[:], in0=mk[:], in1=dut[:], op=A.mult)

        # ---- extract 24 MST weights: 3 rounds of top-8 + match_replace ----
        deaths = sb.tile([B, NT], DT)
        mr1 = sb.tile([B, VV], DT)
        mr2 = sb.tile([B, VV], DT)
        nc.vector.max(deaths[:, 0:8], val[:])
        nc.vector.match_replace(mr1[:], deaths[:, 0:8], val[:], 0.0)
        nc.vector.max(deaths[:, 8:16], mr1[:])
        nc.vector.match_replace(mr2[:], deaths[:, 8:16], mr1[:], 0.0)
        nc.vector.max(deaths[:, 16:24], mr2[:])

        # ---- pre-AR: w' = deaths - 1e-6, w'^2, ONE PE transpose to [64, B]
        # (w'^2 in cols 0:24, w' in cols 32:56 -- partition-aligned blocks) ----
        w12 = sb.tile([B, K2], DT)
        w2 = w12[:, 0:NT]
        w1 = w12[:, 32 : 32 + NT]
        nc.vector.tensor_scalar_add(w1, deaths[:], -GMIN)
        nc.vector.tensor_tensor(out=w2, in0=w1, in1=w1, op=A.mult)
        ps_wT = psum.tile([K2, B], DT)
        with tc.tile_wait_until(0.0705):
            nc.tensor.matmul(out=ps_wT[:], lhsT=w12[:], rhs=id64[:],
                             start=True, stop=True)

        # ---- post-AR: scale lhsT rows (Scalar: activation-with-AP-scale
        # keeps the Vector queue free), then chunked PE/exp/reduce ----
        with tc.tile_wait_until(0.073):
            nc.scalar.mul(LT[0:NT, :], ps_wT[0:NT, :], inv2b[0:NT, 0:1])
            nc.scalar.mul(LT[32 : 32 + NT, :], ps_wT[32 : 32 + NT, :],
                          invb[0:NT, 0:1])
        # PE slices (e 0:32): one PSUM bank (512 floats) per chunk -- a
        # matmul output must not cross a 2KB bank boundary
        EP = 14          # e-channels per PE chunk
        EV0, EV1 = 2 * EP, 50
        ps_s = psum.tile([B, 2, 512], DT)
        fexp = sb.tile([B, E, NT], DT)
        S = sb.tile([B, E], DT)
        # dn = w' * inv on Scalar (activation scale can be a [p,1] AP)
        dn = sb.tile([B, NT], DT)
        with tc.tile_wait_until(0.073):
            nc.scalar.mul(dn[:], w1, invb[:, 0:1])
        for ch in range(2):
            e0, e1 = ch * EP, (ch + 1) * EP
            sview = ps_s[:, ch, 0 : EP * NT].rearrange("p (e q) -> p e q", e=EP)
            nc.tensor.matmul(out=sview, lhsT=LT[:],
                             rhs=rhs[:, e0:e1, :], start=True, stop=True)
            nc.scalar.activation(fexp[:, e0:e1, :], sview, ACT.Exp,
                                 bias=0.0, scale=-1.0)
            nc.vector.tensor_reduce(out=S[:, e0:e1], in_=fexp[:, e0:e1, :],
                                    axis=mybir.AxisListType.X, op=A.add)
        # direct slices: u*(dn - c2)^2 on DVE (fresh tiles: in-place strided
        # ops run ~2x slower) and GpSimd
        tD = sb.tile([B, EV1 - EV0, NT], DT)
        tD2 = sb.tile([B, EV1 - EV0, NT], DT)
        nc.vector.tensor_tensor(
            out=tD[:],
            in0=dn.unsqueeze(1).broadcast_to([B, EV1 - EV0, NT]),
            in1=cu2[:, 0, EV0:EV1].unsqueeze(-1).broadcast_to([B, EV1 - EV0, NT]),
            op=A.subtract,
        )
        nc.vector.tensor_tensor(out=tD2[:], in0=tD[:], in1=tD[:], op=A.mult)
        nc.vector.tensor_tensor(
            out=tD[:], in0=tD2[:],
            in1=cu2[:, 1, EV0:EV1].unsqueeze(-1).broadcast_to([B, EV1 - EV0, NT]),
            op=A.mult,
        )
        nc.scalar.activation(fexp[:, EV0:EV1, :], tD[:], ACT.Exp,
                             bias=0.0, scale=-1.0)
        nc.vector.tensor_reduce(out=S[:, EV0:EV1], in_=fexp[:, EV0:EV1, :],
                                axis=mybir.AxisListType.X, op=A.add)
        tG = sb.tile([B, E - EV1, NT], DT)
        tG2 = sb.tile([B, E - EV1, NT], DT)
        nc.gpsimd.tensor_tensor(
            out=tG[:],
            in0=dn.unsqueeze(1).broadcast_to([B, E - EV1, NT]),
            in1=cu2[:, 0, EV1:E].unsqueeze(-1).broadcast_to([B, E - EV1, NT]),
            op=A.subtract,
        )
        nc.gpsimd.tensor_tensor(out=tG2[:], in0=tG[:], in1=tG[:], op=A.mult)
        nc.gpsimd.tensor_tensor(
            out=tG[:], in0=tG2[:],
            in1=cu2[:, 1, EV1:E].unsqueeze(-1).broadcast_to([B, E - EV1, NT]),
            op=A.mult,
        )
        nc.scalar.activation(fexp[:, EV1:E, :], tG[:], ACT.Exp,
                             bias=0.0, scale=-1.0)
        nc.vector.tensor_reduce(out=S[:, EV1:E], in_=fexp[:, EV1:E, :],
                                axis=mybir.AxisListType.X, op=A.add)
        outt = sb.tile([B, E], DT)
        nc.vector.tensor_tensor(out=outt[:], in0=S[:], in1=Ab[:], op=A.mult)
        nc.sync.dma_start(out_d[:], outt[:])

    _split_excess_waits(nc)
    return nc


_CACHE = {}


def _consts():
    # pair matrix: adds partition rows b and b+64 (the two T-halves) and
    # applies the 1/T mean scale
    pairmat = np.zeros((128, B), dtype=np.float32)
    for p in range(128):
        pairmat[p, p % B] = 1.0 / T
    ut = np.broadcast_to(
        np.triu(np.ones((V, V), dtype=np.float32), k=1).reshape(1, VV), (B, VV)
    ).copy()
    return pairmat, np.ascontiguousarray(ut), np.eye(B, dtype=np.float32)


def _param_consts(centres, sharpness):
    """Parameter-only preprocessing (host): the structure-element rhs
    [K2, E*NT] for the K2-deep PE contraction, and the pre-broadcast
    Ab = exp(-(s1*c1)^2) factor.

      s[b,(e,p)] = inv^2*w2[b,p]*u_e - 2*inv*w1[b,p]*(u*c2)_e + (u*c2^2)_e
      rows 0:24  = delta(p==k) * u_e        (paired with LT w'^2 block)
      rows 32:56 = delta(p==k) * (-2*u*c2)_e (paired with LT w' block)
      row  56    = (u*c2^2)_e                (paired with LT ones row)
    """
    u = (sharpness[:, 1] ** 2).astype(np.float32)
    c2 = centres[:, 1].astype(np.float32)
    rhs = np.zeros((K2, E, NT), dtype=np.float32)
    for k in range(NT):
        rhs[k, :, k] = u
        rhs[32 + k, :, k] = -2.0 * u * c2
    rhs[ROW_Q] = (u * c2 * c2)[:, None]
    ab = np.exp(-((sharpness[:, 0] * centres[:, 0]) ** 2)).astype(np.float32)
    ab_b = np.broadcast_to(ab[None, :], (B, E)).copy()
    cu = np.concatenate([c2, u]).astype(np.float32).reshape(1, 2 * E)
    return np.ascontiguousarray(rhs.reshape(K2, E * NT)), ab_b, np.ascontiguousarray(cu)


def _get_program():
    if "nc" not in _CACHE:
        _CACHE["nc"] = _build_program()
    return _CACHE["nc"]


def _run(x, centres, sharpness, **run_kwargs):
    nc = _get_program()
    xf = np.ascontiguousarray(x.reshape(-1, C, T, V)).astype(np.float32, copy=False)
    n_total = xf.shape[0]
    assert n_total == N_CORES * B, xf.shape
    pairmat, ut, id64 = _consts()
    rhs, ab, cu = _param_consts(np.asarray(centres), np.asarray(sharpness))
    in_maps = [
        {
            "x": np.ascontiguousarray(xf[i * B : (i + 1) * B]),
            "pm": pairmat,
            "ut": ut,
            "id64": id64,
            "rhs": rhs,
            "ab": ab,
            "cu": cu,
        }
        for i in range(N_CORES)
    ]
    res = run_bass_kernel_spmd(nc, in_maps, list(range(N_CORES)), **run_kwargs)
    out = np.concatenate([res.results[i]["out"] for i in range(N_CORES)], axis=0)
    return out, res


def kernel(x, centres, sharpness):
    out, _ = _run(np.asarray(x), np.asarray(centres), np.asarray(sharpness))
    return out
